# revision 26
# baseline (speedup 1.0000x reference)
"""Distributed Trainium2 kernel for nn_Attention_64742337020012.

B=4, N=2048, E=768, H=12, D=64 causal attention with per-head RMS norm,
interleaved xpos RoPE, and output projection.

Sharding: 8 cores, core c owns batch c//2 and heads 6*(c%2) .. 6*(c%2)+6
(head-independent attention).  Each core computes full causal attention for
its 6 heads over all 2048 positions plus the partial output projection using
its 384 rows of proj_w^T; the host sums the two partial projections per batch
and adds the bias.

Pipeline (all matmuls bf16, f32 accumulation):
  1. q,k loaded bf16 (cast on host), roped via host-precomputed coefficient
     tables (head dim pre-permuted evens-first); rsqrt(ms) via Quake seed +
     1 Newton step on DVE batched across 4-tile groups.  Rope/norm bulk ops
     are 4-tile fused on GpSimd; latency-critical small ops stay on DVE.
  2. q',k' transposed to [d, n] via PE transposes into 2 rotating PSUM slot
     TRIPLES; each triple drains with a single [P,3,128] DVE copy.
  3. Scores computed transposed (S^T[k,q] tiles), causal-trimmed, exp on ACT
     straight out of PSUM, paired triangular-mask multiplies on DVE.
  4. AV with P^T stationary and [V | 1] moving gives y and the softmax
     denominator in one accumulation; per-row reciprocal normalizes.
     PE stream software-pipelined one head deep with AV/proj filler.
  5. y transposed on PE, projected in 3x256-col chunks, bf16 partials
     written to DRAM (host sums the two per-batch partials in f32).
Startup is latency-tuned: per-tile q/k group-0 DMAs ahead of the rope
tables (split so tiles 0-3's tables land first), tile-0 norm fast path,
bulk v/wt/tabs loads deferred past the startup-critical DMAs.
"""

import sys

sys.path.insert(0, "/opt/trn_rl_repo")

import numpy as np
import ml_dtypes

import concourse.bass as bass
import concourse.mybir as mybir
import concourse.tile as tile
from concourse.bass_utils import run_bass_kernel_spmd

# ----------------------------------------------------------------------------
# Workaround for this container's walrus build: the TileContext tail drain
# carries one SyncWait per outstanding semaphore, but CoreV3 CTRL codegen
# accepts only a single sync wait per instruction.  Split the waits across
# single-wait NOPs emitted right after the drain.
from concourse.vector_clock import ScopedClock as _ScopedClock


def _split_sync_waits(nc, inst, max_waits=1):
    si = inst.ins.sync_info
    if si is None:
        return
    waits = list(si.on_wait)
    if len(waits) <= max_waits:
        return
    inst.ins.sync_info = mybir.SyncInfo(
        on_wait=waits[:max_waits], on_update=list(si.on_update)
    )
    for i in range(max_waits, len(waits), max_waits):
        nop = nc.sync.nop(nofuse=True, hint="drain_wait_split")
        nop.ins.sync_info = mybir.SyncInfo(
            on_wait=waits[i : i + max_waits], on_update=[]
        )


def _patched_drain_and_barrier(self, tick_clock, wait_clock):
    nc = self.nc
    drain_inst = nc.sync.drain()
    wait_clock.add_sem_waits(
        drain_inst.ins, _ScopedClock({None: tick_clock.global_clock})
    )
    _split_sync_waits(nc, drain_inst)
    nc.all_engine_barrier()
    assert self.sems is not None
    popped = nc._tile_sem_poison_stack.pop()
    assert popped is self._sem_poison
    nc.clear_and_free_semaphores(list(self.sems.allocated().values()))
    nc.all_engine_barrier()


tile.TileContext._drain_and_barrier = _patched_drain_and_barrier


# Same walrus limitation, applied globally: any instruction carrying more
# than one SyncWait gets the extra waits hoisted onto same-engine NoOps
# inserted immediately before it in the BIR json.
import json as _json
import concourse.bass2jax as _bass2jax

_orig_compile_bir_kernel = _bass2jax.compile_bir_kernel


def _split_waits_in_bir(bir_json: bytes) -> bytes:
    j = _json.loads(bir_json)
    n_new = [0]
    for fn in j["functions"]:
        for bb in fn["blocks"]:
            insts = bb["instructions"]
            out = []
            for inst in insts:
                si = inst.get("sync_info")
                waits = (si or {}).get("on_wait") or []
                if len(waits) > 1:
                    for w in waits[:-1]:
                        n_new[0] += 1
                        out.append({
                            "engine": inst["engine"],
                            "ins": [], "outs": [],
                            "name": f"{inst['name']}-ws{n_new[0]}",
                            "opcode": "NoOp",
                            "sync_info": {"on_wait": [w], "on_update": []},
                        })
                    si["on_wait"] = [waits[-1]]
                out.append(inst)
            bb["instructions"] = out
    return _json.dumps(j).encode()


def _patched_compile_bir_kernel(bir_json, tmpdir, neff_name="file.neff"):
    return _orig_compile_bir_kernel(_split_waits_in_bir(bir_json), tmpdir, neff_name)


_bass2jax.compile_bir_kernel = _patched_compile_bir_kernel
# ----------------------------------------------------------------------------

B, N, E, H = 4, 2048, 768, 12
D = 64
RDIM = 32
EPS = 1e-6
XPOS_SCALE_BASE = 512.0
THETA = 10000.0

HL = 6            # heads per core
EL = HL * D       # 384 local embed cols
P = 128
NT = N // P       # 16 row tiles
QC = 512          # q chunk (columns of S^T tiles)
NQC = N // QC     # 4
TW = 2 * (D + RDIM)   # tabs width per tile (k/q cos+sin) = 192
F32 = mybir.dt.float32
BF16 = mybir.dt.bfloat16
I32 = mybir.dt.int32

_CACHE = {}


def _head_perm():
    """Per-head column permutation: rotary evens, rotary odds, passthrough."""
    p = list(range(0, RDIM, 2)) + list(range(1, RDIM, 2)) + list(range(RDIM, D))
    return np.array(p, dtype=np.int64)


def _build_tables(scale_vec, invert_xpos):
    """cosPt [N, 64], sinPt [N, 32] coefficient tables in permuted layout."""
    inv_freq = 1.0 / (THETA ** (np.arange(0, RDIM, 2, dtype=np.float64) / RDIM))
    t = np.arange(N, dtype=np.float64)
    freqs = t[:, None] * inv_freq[None, :]           # [N, 16]
    cos0, sin0 = np.cos(freqs), np.sin(freqs)
    base = (np.arange(0, RDIM, 2, dtype=np.float64) + 0.4 * RDIM) / (1.4 * RDIM)
    power = (t - N // 2) / XPOS_SCALE_BASE
    xsc = base[None, :] ** power[:, None]            # [N, 16]
    if invert_xpos:
        xsc = 1.0 / xsc
    sc = np.asarray(scale_vec, dtype=np.float64)
    cosPt = np.empty((N, D), dtype=np.float64)
    sinPt = np.empty((N, RDIM), dtype=np.float64)
    i = np.arange(16)
    cosPt[:, 0:16] = cos0 * xsc * sc[2 * i][None, :]
    cosPt[:, 16:32] = cos0 * xsc * sc[2 * i + 1][None, :]
    cosPt[:, 32:] = sc[RDIM:][None, :]
    sinPt[:, 0:16] = -sin0 * xsc * sc[2 * i + 1][None, :]
    sinPt[:, 16:32] = sin0 * xsc * sc[2 * i][None, :]
    return cosPt.astype(np.float32), sinPt.astype(np.float32)


def build_graph():
    nc = bass.Bass()
    q_ext = nc.declare_dram_parameter("q", [N, EL], BF16, isOutput=False)
    k_ext = nc.declare_dram_parameter("k", [N, EL], BF16, isOutput=False)
    v_ext = nc.declare_dram_parameter("v", [P, NT * HL * (D + 1)], BF16, isOutput=False)
    wt_ext = nc.declare_dram_parameter("wt", [P, 3 * E], BF16, isOutput=False)
    tabs_ext = nc.declare_dram_parameter(
        "tabs", [P, NT * TW], BF16, isOutput=False)
    tri_ext = nc.declare_dram_parameter("tri", [P, P], BF16, isOutput=False)
    out_ext = nc.declare_dram_parameter("out", [N, E], BF16, isOutput=True)

    q_t4 = q_ext.rearrange("(g t p) e -> g p t e", t=4, p=P)
    k_t4 = k_ext.rearrange("(g t p) e -> g p t e", t=4, p=P)
    out_t = out_ext.rearrange("(t p) e -> t p e", p=P)

    with tile.TileContext(nc) as tc:
        with (
            tc.tile_pool(name="persist", bufs=1) as persist,
            tc.tile_pool(name="qk_in", bufs=3) as qk_in,
            tc.tile_pool(name="pp", bufs=2) as pp,
            tc.tile_pool(name="pp_small", bufs=2) as pp_small,
            tc.tile_pool(name="pt_pool", bufs=22) as pt_pool,
            tc.tile_pool(name="ypre", bufs=3) as ypre_pool,
            tc.tile_pool(name="yt_pool", bufs=3) as yt_pool,
            tc.tile_pool(name="recip", bufs=8) as recip_pool,
            tc.tile_pool(name="outsb", bufs=4) as outsb_pool,
            tc.tile_pool(name="ps_s", bufs=2, space="PSUM") as ps_s,
            tc.tile_pool(name="ps_y", bufs=2, space="PSUM") as ps_y,
            tc.tile_pool(name="ps_t", bufs=1, space="PSUM") as ps_t,
            tc.tile_pool(name="ps_o", bufs=1, space="PSUM") as ps_o,
        ):
            # ---------------- constants (host-prepared layouts) ----------------
            ident = persist.tile([P, P], BF16)
            from concourse.masks import make_identity
            make_identity(nc, ident)
            tabs_sb = persist.tile([P, NT, 2, D + RDIM], BF16)
            tri_sb = persist.tile([P, P], BF16)
            wt_sb = persist.tile([P, 3, E], BF16)
            vall = persist.tile([P, NT, HL, D + 1], BF16)

            # transposed q', k': [128 = 2-head d, hp, n]
            qT = persist.tile([P, 3, N], BF16, name="qT")
            kT = persist.tile([P, 3, N], BF16, name="kT")

            # rotating PSUM transpose slot triples: 2 x [P, 3, 128] bf16
            psT = ps_t.tile([P, 6, P], BF16, name="psT")
            slot_ctr = [0]

            def next_slot3():
                b = (slot_ctr[0] % 2) * 3
                slot_ctr[0] += 1
                return b

            def ap4(t, offset, dims):
                return bass.AP(tensor=t.tensor, offset=t.offset + offset,
                               ap=[t.ap[0]] + dims)

            # ---------------- preprocess helpers ----------------
            def rstd_chain(ssum, nt, tag):
                """ssum [P, nt, 2HL] f32 -> rsqrt(ssum) (DVE, Quake+Newton).

                rsqrt(ssum_k) = 0.125*rstd_k (folds 1/sqrt(D) for scores);
                rsqrt(ssum_q) = 0.125*rstd_q, with the 8x folded into the
                host-side q rope tables.  eps is dropped: ssum ~ chi2(64)
                is bounded well away from 0 for these inputs.
                """
                ish = pp_small.tile([P, nt, 2 * HL], I32, tag=f"i{tag}", name="ish")
                nc.vector.tensor_scalar(out=ish[:], in0=ssum.bitcast(I32),
                                        scalar1=1, scalar2=None,
                                        op0=mybir.AluOpType.logical_shift_right)
                y0i = pp_small.tile([P, nt, 2 * HL], I32, tag=f"y{tag}", name="y0i")
                nc.vector.tensor_scalar(out=y0i[:], in0=ish[:],
                                        scalar1=-1, scalar2=0x5F3759DF,
                                        op0=mybir.AluOpType.mult,
                                        op1=mybir.AluOpType.add)
                y = y0i.bitcast(F32)
                rstd = pp_small.tile([P, nt, 2 * HL], F32, tag=f"r{tag}", name="rstd")
                t_nr = pp_small.tile([P, nt, 2 * HL], F32, tag=f"t{tag}", name="t_nr")
                nc.vector.tensor_mul(t_nr[:], y, y)
                nc.vector.tensor_mul(t_nr[:], t_nr[:], ssum[:])
                nc.vector.tensor_scalar(out=t_nr[:], in0=t_nr[:], scalar1=-0.5,
                                        scalar2=1.5, op0=mybir.AluOpType.mult,
                                        op1=mybir.AluOpType.add)
                nc.vector.tensor_mul(rstd[:], y, t_nr[:])
                return rstd

            def rope_side(xg, t0, nt, i0, side, rstd, rstd_t0,
                          eng_big, eng_small):
                """Rope+norm for group tiles t0..t0+nt of one side.

                Writes a [P, 4, HL, D] tile's [t0:t0+nt] slice and returns it.
                xg [P, 2, 4, HL, D]; tables rows i0..i0+nt; rstd [P, *, 2HL]
                with tile offset rstd_t0.  eng_big runs the two [*,D]-wide
                muls, eng_small the three [*,16/32]-wide rotation ops.
                """
                tag = "kq"[side]
                xh = ap4(xg, side * 4 * HL * D + t0 * HL * D,
                         [[HL * D, nt], [D, HL], [1, D]])
                cosb = ap4(tabs_sb, i0 * TW + side * (TW // 2),
                           [[TW, nt], [0, HL], [1, D]])
                pre = pp.tile([P, 4, HL, D], BF16, tag=f"pre{tag}", name="pre")
                pre_s = ap4(pre, t0 * HL * D, [[HL * D, nt], [D, HL], [1, D]])
                eng_big.tensor_mul(pre_s, xh, cosb)
                tmp = pp.tile([P, 4, HL, RDIM], BF16, tag=f"tmp{tag}", name="tmp")
                xh_odd = ap4(xg, side * 4 * HL * D + t0 * HL * D + 16,
                             [[HL * D, nt], [D, HL], [1, 16]])
                xh_evn = ap4(xg, side * 4 * HL * D + t0 * HL * D,
                             [[HL * D, nt], [D, HL], [1, 16]])
                sin_lo = ap4(tabs_sb, i0 * TW + side * (TW // 2) + D,
                             [[TW, nt], [0, HL], [1, 16]])
                sin_hi = ap4(tabs_sb, i0 * TW + side * (TW // 2) + D + 16,
                             [[TW, nt], [0, HL], [1, 16]])
                tmp_lo = ap4(tmp, t0 * HL * RDIM,
                             [[HL * RDIM, nt], [RDIM, HL], [1, 16]])
                tmp_hi = ap4(tmp, t0 * HL * RDIM + 16,
                             [[HL * RDIM, nt], [RDIM, HL], [1, 16]])
                eng_small.tensor_mul(tmp_lo, xh_odd, sin_lo)
                eng_small.tensor_mul(tmp_hi, xh_evn, sin_hi)
                pre_rot = ap4(pre, t0 * HL * D, [[HL * D, nt], [D, HL], [1, RDIM]])
                tmp_s = ap4(tmp, t0 * HL * RDIM,
                            [[HL * RDIM, nt], [RDIM, HL], [1, RDIM]])
                eng_small.tensor_add(pre_rot, pre_rot, tmp_s)
                a = pp.tile([P, 4, HL, D], BF16, tag=f"a{tag}", name="a")
                a_s = ap4(a, t0 * HL * D, [[HL * D, nt], [D, HL], [1, D]])
                rstd_b = ap4(rstd, rstd_t0 * 2 * HL + side * HL,
                             [[2 * HL, nt], [1, HL], [0, D]])
                eng_big.tensor_mul(a_s, pre_s, rstd_b)
                return a

            def emit_tp(a, t_local, i_global, dstT):
                b = next_slot3()
                for hp in range(3):
                    nc.tensor.transpose(
                        psT[:, b + hp, :],
                        a[:, t_local, 2 * hp: 2 * hp + 2, :], ident[:])
                nc.vector.tensor_copy(
                    dstT[:, :, i_global * P:(i_global + 1) * P],
                    psT[:, b:b + 3, :])

            def preprocess_startup():
                """Group 0 with per-tile DMAs and a tile-0 latency fast path."""
                xg = qk_in.tile([P, 2, 4, HL, D], BF16, tag="xqk", name="x0")
                # tile 0 + its rope tables first (startup critical path),
                # then the remaining tiles
                nc.sync.dma_start(out=xg[:, 0, 0], in_=k_t4[0][:, 0])
                nc.sync.dma_start(out=xg[:, 1, 0], in_=q_t4[0][:, 0])
                nc.sync.dma_start(out=tabs_sb[:, 0:4], in_=tabs_ext[:, 0:4 * TW])
                for ii in range(1, 4):
                    nc.sync.dma_start(out=xg[:, 0, ii], in_=k_t4[0][:, ii])
                    nc.sync.dma_start(out=xg[:, 1, ii], in_=q_t4[0][:, ii])
                nc.sync.dma_start(out=tri_sb[:], in_=tri_ext[:])
                # independent per-DMA-arrival chains: tile 0, tiles 1-2,
                # tile 3 — each tile's transposes flow as its data lands
                def chain(t_lo, nt, tag, ke=nc.gpsimd, qe=nc.vector):
                    sq = pp.tile([P, 2, nt, HL, D], BF16, tag=f"sq{tag}",
                                 name=f"sq{tag}")
                    for s in range(2):
                        nc.scalar.activation(
                            out=sq[:, s], in_=xg[:, s, t_lo:t_lo + nt],
                            func=mybir.ActivationFunctionType.Square)
                    ss = pp_small.tile([P, nt, 2 * HL], F32, tag=f"ss{tag}",
                                       name=f"ss{tag}")
                    nc.vector.reduce_sum(
                        ap4(ss, 0, [[2 * HL, nt], [1, HL]]), sq[:, 0],
                        axis=mybir.AxisListType.X)
                    nc.vector.reduce_sum(
                        ap4(ss, HL, [[2 * HL, nt], [1, HL]]), sq[:, 1],
                        axis=mybir.AxisListType.X)
                    rstd = rstd_chain(ss, nt, tag)
                    ak = rope_side(xg, t_lo, nt, t_lo, 0, rstd, 0, ke, ke)
                    for t in range(nt):
                        emit_tp(ak, t_lo + t, t_lo + t, kT)
                    aq = rope_side(xg, t_lo, nt, t_lo, 1, rstd, 0, qe, qe)
                    for t in range(nt):
                        emit_tp(aq, t_lo + t, t_lo + t, qT)

                chain(0, 1, "0")
                chain(1, 2, "12")
                chain(3, 1, "3", ke=nc.vector, qe=nc.gpsimd)

            def preprocess_group(i0):
                """Batched 4-tile group load + norm stats (GpSimd bulk)."""
                g = i0 // 4
                xg = qk_in.tile([P, 2, 4, HL, D], BF16, tag="xqk", name="x")
                nc.sync.dma_start(out=xg[:, 0], in_=k_t4[g])
                nc.sync.dma_start(out=xg[:, 1], in_=q_t4[g])
                sq = pp.tile([P, 2, 4, HL, D], BF16, tag="sq", name="sq")
                for s in range(2):
                    nc.scalar.activation(
                        out=sq[:, s], in_=xg[:, s],
                        func=mybir.ActivationFunctionType.Square)
                ssum = pp_small.tile([P, 4, 2 * HL], F32, tag="ss4", name="ssum")
                nc.vector.reduce_sum(
                    ap4(ssum, 0, [[2 * HL, 4], [1, HL]]), sq[:, 0],
                    axis=mybir.AxisListType.X)
                nc.vector.reduce_sum(
                    ap4(ssum, HL, [[2 * HL, 4], [1, HL]]), sq[:, 1],
                    axis=mybir.AxisListType.X)
                rstd = rstd_chain(ssum, 4, "4")
                return xg, rstd

            def finish_group_side(grp, i0, side):
                """Rope+norm+transpose all 4 tiles of one side (GpSimd bulk)."""
                xg, rstd = grp
                a = rope_side(xg, 0, 4, i0, side, rstd, 0,
                              nc.gpsimd, nc.gpsimd)
                dstT = (kT, qT)[side]
                for t in range(4):
                    emit_tp(a, t, i0 + t, dstT)

            # ---------------- attention emit helpers ----------------
            def emit_qk_exp(h, qc, filler, split_q=False):
                """Scores + exp + diag mask for (head, q-chunk). Returns pt list."""
                hp, hh = divmod(h, 2)
                hoff = 64 * hh
                nkt = 4 * (qc + 1)
                ngroups = nkt // 2
                pts = []
                for kg in range(ngroups):
                    ss = ps_s.tile([P, 2, QC], F32, tag="ps_s", name="ss")
                    for j in range(2):
                        kt = kg * 2 + j
                        doff = kt - 4 * qc
                        qstart = max(doff, 0) * P
                        if split_q:
                            # per-q-tile matmuls so the first scores start as
                            # soon as each transposed tile lands (startup)
                            for qt in range(qstart // P, 4):
                                diag = qt == doff
                                nc.tensor.matmul(
                                    ss[:, j, qt * P:(qt + 1) * P],
                                    kT[hoff:hoff + 64, hp, kt * P:(kt + 1) * P],
                                    qT[hoff:hoff + 64, hp,
                                       qc * QC + qt * P: qc * QC + (qt + 1) * P],
                                    start=True, stop=not diag,
                                )
                                if diag:
                                    # causal mask: -1e30 bias closes the group
                                    nc.tensor.matmul(
                                        ss[:, j, qt * P:(qt + 1) * P],
                                        ident[:], tri_sb[:],
                                        start=False, stop=True,
                                    )
                            continue
                        nc.tensor.matmul(
                            ss[:, j, qstart:QC],
                            kT[hoff : hoff + 64, hp, kt * P : (kt + 1) * P],
                            qT[hoff : hoff + 64, hp, qc * QC + qstart : (qc + 1) * QC],
                            start=True, stop=doff < 0,
                        )
                        if doff >= 0:
                            # causal mask as -1e30 bias on the diagonal block,
                            # same accumulation group as the score matmul (no
                            # DVE dependency on the exp->AV path)
                            nc.tensor.matmul(
                                ss[:, j, doff * P:(doff + 1) * P],
                                ident[:], tri_sb[:],
                                start=False, stop=True,
                            )
                    pt = pt_pool.tile([P, 2, QC], BF16, tag="pt", name="pt")
                    # exp only the causally-live q range (union over the pair)
                    qmin = max(kg * 2 - 4 * qc, 0) * P
                    nc.scalar.activation(
                        out=pt[:, :, qmin:], in_=ss[:, :, qmin:],
                        func=mybir.ActivationFunctionType.Exp,
                    )
                    pts.append(pt)
                    # interleave prev head's AV units as PE filler
                    if filler:
                        n = -(-len(filler) // (ngroups - kg))
                        for _ in range(min(n, len(filler))):
                            filler.pop(0)()
                return pts

            def make_av_units(h, qc, pts, ypre3):
                """Per-kt AV emission closures + final normalize closure."""
                hp, hh = divmod(h, 2)
                nkt = 4 * (qc + 1)
                ys = ps_y.tile([P, 4, D + 1], F32, tag="ps_y", name="ys")

                def mk(kt):
                    def unit():
                        pt = pts[kt // 2]
                        j = kt % 2
                        for qs in range(4):
                            first = kt == 0 and qs == 0
                            last = kt == nkt - 1 and qs == 3
                            if (not first and not last
                                    and kt * P >= qc * QC + (qs + 1) * P):
                                continue
                            nc.tensor.matmul(
                                ys[:, qs, :],
                                pt[:, j, qs * P : (qs + 1) * P],
                                vall[:, kt, h, :],
                                start=first, stop=last,
                            )
                    return unit

                def fin():
                    r = recip_pool.tile([P, 4], F32, tag="recip", name="r")
                    nc.vector.reciprocal(out=r[:], in_=ys[:, :, D])
                    r_b = bass.AP(tensor=r.tensor, offset=r.offset,
                                  ap=[r.ap[0], r.ap[1], [0, D]])
                    nc.vector.tensor_mul(
                        ypre3[hp][:, :, hh, :], ys[:, :, 0:D], r_b)

                return [mk(kt) for kt in range(nkt)] + [fin]

            def proj_subunits(qc, ypre3, qs):
                """One row tile's projection as 4 filler closures."""
                nt_i = qc * 4 + qs
                box = {}

                def t_unit():
                    box["yt"] = yt_pool.tile([P, 3, P], BF16, tag="yt",
                                             name="yt")
                    box["osb"] = outsb_pool.tile([P, E], BF16, tag="osb",
                                                 name="osb")
                    box["po"] = ps_o.tile([P, 2, 256], F32, tag="ps_o",
                                          name="po")
                    b = next_slot3()
                    for hp in range(3):
                        nc.tensor.transpose(
                            psT[:, b + hp, :], ypre3[hp][:, qs, :, :], ident[:]
                        )
                    nc.vector.tensor_copy(box["yt"][:, :, :], psT[:, b:b + 3, :])

                def o_unit(oh):
                    def u():
                        yt, osb, po = box["yt"], box["osb"], box["po"]
                        sl = oh % 2
                        for ec in range(3):
                            nc.tensor.matmul(
                                po[:, sl, :],
                                yt[:, ec, :],
                                wt_sb[:, ec, oh * 256 : (oh + 1) * 256],
                                start=(ec == 0), stop=(ec == 2),
                            )
                        nc.vector.tensor_copy(
                            osb[:, oh * 256 : (oh + 1) * 256], po[:, sl, :]
                        )
                        if oh == 2:
                            nc.sync.dma_start(out=out_t[nt_i], in_=osb[:])
                    return u

                return [t_unit, o_unit(0), o_unit(1), o_unit(2)]

            # ---------------- main pipelined schedule ----------------
            pending_av = []     # at most one (h, qc, pts, ypre3)
            pending_proj = []   # (qc, ypre3, qs) row-tile units
            preprocess_startup()
            for qc in range(NQC):
                ypre3_cur = [
                    ypre_pool.tile([P, 4, 2, D], BF16, tag=f"ypre{_i}",
                                   name=f"ypre{_i}")
                    for _i in range(3)
                ]
                for h in range(HL):
                    if qc == 0 and h == 1:
                        # bulk loads after the startup-critical q/k DMAs:
                        # v first (first AV filler), then the remaining rope
                        # tables (group-1 finish), then weights (projection)
                        nc.sync.dma_start(out=vall[:], in_=v_ext[:])
                        nc.sync.dma_start(out=tabs_sb[:, 4:NT],
                                          in_=tabs_ext[:, 4 * TW:])
                    filler = []
                    meta = None
                    if pending_av:
                        ph, pqc, ppts, pypre = pending_av.pop()
                        filler = make_av_units(ph, pqc, ppts, pypre)
                        meta = (ph, pqc, pypre)
                    for _ in range(5):
                        if pending_proj:
                            filler.append(pending_proj.pop(0))
                    pts = emit_qk_exp(h, qc, filler,
                                      split_q=(qc == 0 and h == 0))
                    for u in filler:
                        u()
                    if meta is not None:
                        ph, pqc, pypre = meta
                        if ph == HL - 1:
                            for qs in range(4):
                                pending_proj.extend(
                                    proj_subunits(pqc, pypre, qs))
                    pending_av.append((h, qc, pts, ypre3_cur))
                    # preprocess next q-chunk's tiles inside the head loop,
                    # k/q sides split across head windows to avoid bursts
                    # (qc0 starts at h==1 to keep startup uncongested)
                    if qc < NQC - 1:
                        hofs = 1 if qc == 0 else 0
                        if h == hofs:
                            grp = preprocess_group(4 * (qc + 1))
                        elif h == hofs + 1:
                            if qc == 0:
                                nc.sync.dma_start(out=wt_sb[:], in_=wt_ext[:])
                            finish_group_side(grp, 4 * (qc + 1), 0)
                        elif h == hofs + 2:
                            finish_group_side(grp, 4 * (qc + 1), 1)
            # drain
            ph, pqc, ppts, pypre = pending_av.pop()
            for u in make_av_units(ph, pqc, ppts, pypre):
                u()
            while pending_proj:
                pending_proj.pop(0)()
            for qs in range(4):
                for u in proj_subunits(pqc, pypre, qs):
                    u()
    return nc


def _get_graph():
    if "nc" not in _CACHE:
        _CACHE["nc"] = build_graph()
    return _CACHE["nc"]


def _host_inputs(q, k, v, q_scale, k_scale, proj_w):
    perm = _head_perm()
    bf = ml_dtypes.bfloat16
    qcos, qsin = _build_tables(q_scale, invert_xpos=False)
    kcos, ksin = _build_tables(k_scale, invert_xpos=True)
    # device computes rsqrt(ssum_q) = 0.125*rstd_q; fold the 8x here
    qcos, qsin = qcos * 8.0, qsin * 8.0

    def tab_layout(t):
        # [N, w] -> [128, NT*w] matching sbuf tile [P, NT, w]
        w = t.shape[1]
        return np.ascontiguousarray(
            t.reshape(NT, P, w).transpose(1, 0, 2).reshape(P, NT * w)).astype(bf)

    qcos_r, qsin_r = tab_layout(qcos), tab_layout(qsin)
    kcos_r, ksin_r = tab_layout(kcos), tab_layout(ksin)
    # combined [P, NT, 2, D+RDIM]: slot 0 = k tables, slot 1 = q tables
    tabs = np.empty((P, NT, 2, D + RDIM), dtype=kcos_r.dtype)
    tabs[:, :, 0, 0:D] = kcos_r.reshape(P, NT, D)
    tabs[:, :, 0, D:] = ksin_r.reshape(P, NT, RDIM)
    tabs[:, :, 1, 0:D] = qcos_r.reshape(P, NT, D)
    tabs[:, :, 1, D:] = qsin_r.reshape(P, NT, RDIM)
    tabs_r = np.ascontiguousarray(tabs.reshape(P, NT * TW))
    # additive causal mask for the diagonal [128,128] sub-block:
    # tri[k, q] = 0 where q >= k else -1e30 (exp -> exactly 0)
    tri = np.where(np.arange(P)[None, :] >= np.arange(P)[:, None],
                   0.0, -1e30).astype(np.float32)
    tri_r = np.ascontiguousarray(tri).astype(bf)

    in_maps = []
    for c in range(8):
        b = c // 2
        h0 = HL * (c % 2)
        cols = np.concatenate([(h0 + h) * D + perm for h in range(HL)])
        vcols = np.arange(h0 * D, (h0 + HL) * D)
        v_aug = np.ones((N, HL, D + 1), np.float32)
        v_aug[:, :, :D] = v[b][:, vcols].reshape(N, HL, D)
        wt_l = np.ascontiguousarray(proj_w[:, vcols].T)   # [384, 768]
        wt_r = np.ascontiguousarray(
            wt_l.reshape(3, P, E).transpose(1, 0, 2).reshape(P, 3 * E))
        in_maps.append({
            "q": np.ascontiguousarray(q[b][:, cols]).astype(bf),
            "k": np.ascontiguousarray(k[b][:, cols]).astype(bf),
            "v": np.ascontiguousarray(
                v_aug.reshape(NT, P, HL * (D + 1)).transpose(1, 0, 2)
                .reshape(P, NT * HL * (D + 1))).astype(bf),
            "wt": wt_r.astype(bf),
            "tabs": tabs_r,
            "tri": tri_r,
        })
    return in_maps


def kernel(q, k, v, q_scale, k_scale, proj_w, proj_b):
    nc = _get_graph()
    in_maps = _host_inputs(q, k, v, q_scale, k_scale, proj_w)
    res = run_bass_kernel_spmd(nc, in_maps, list(range(8)))
    out = np.empty((B, N, E), np.float32)
    for b in range(B):
        out[b] = (res.results[2 * b]["out"].astype(np.float32)
                  + res.results[2 * b + 1]["out"].astype(np.float32))
    out += proj_b[None, None, :].astype(np.float32)
    return out


# revision 27
# speedup vs baseline: 1.0229x; 1.0229x over previous
"""Distributed Trainium2 kernel for nn_Attention_64742337020012.

B=4, N=2048, E=768, H=12, D=64 causal attention with per-head RMS norm,
interleaved xpos RoPE, and output projection.

Sharding: 8 cores, core c owns batch c//2 and heads 6*(c%2) .. 6*(c%2)+6
(head-independent attention).  Each core computes full causal attention for
its 6 heads over all 2048 positions plus the partial output projection using
its 384 rows of proj_w^T; the host sums the two partial projections per batch
and adds the bias.

Pipeline (all matmuls bf16, f32 accumulation):
  1. q,k loaded bf16 (cast on host), roped via host-precomputed coefficient
     tables (head dim pre-permuted evens-first); rsqrt(ms) via Quake seed +
     1 Newton step on DVE batched across 4-tile groups.  Rope/norm bulk ops
     are 4-tile fused on GpSimd; latency-critical small ops stay on DVE.
  2. q',k' transposed to [d, n] via PE transposes into 2 rotating PSUM slot
     TRIPLES; each triple drains with a single [P,3,128] DVE copy.
  3. Scores computed transposed (S^T[k,q] tiles), causal-trimmed, exp on ACT
     straight out of PSUM, paired triangular-mask multiplies on DVE.
  4. AV with P^T stationary and [V | 1] moving gives y and the softmax
     denominator in one accumulation; per-row reciprocal normalizes.
     PE stream software-pipelined one head deep with AV/proj filler.
  5. y transposed on PE, projected in 3x256-col chunks, bf16 partials
     written to DRAM (host sums the two per-batch partials in f32).
Startup is latency-tuned: per-tile q/k group-0 DMAs ahead of the rope
tables (split so tiles 0-3's tables land first), tile-0 norm fast path,
bulk v/wt/tabs loads deferred past the startup-critical DMAs.
"""

import sys

sys.path.insert(0, "/opt/trn_rl_repo")

import numpy as np
import ml_dtypes

import concourse.bass as bass
import concourse.mybir as mybir
import concourse.tile as tile
from concourse.bass_utils import run_bass_kernel_spmd

# ----------------------------------------------------------------------------
# Workaround for this container's walrus build: the TileContext tail drain
# carries one SyncWait per outstanding semaphore, but CoreV3 CTRL codegen
# accepts only a single sync wait per instruction.  Split the waits across
# single-wait NOPs emitted right after the drain.
from concourse.vector_clock import ScopedClock as _ScopedClock


def _split_sync_waits(nc, inst, max_waits=1):
    si = inst.ins.sync_info
    if si is None:
        return
    waits = list(si.on_wait)
    if len(waits) <= max_waits:
        return
    inst.ins.sync_info = mybir.SyncInfo(
        on_wait=waits[:max_waits], on_update=list(si.on_update)
    )
    for i in range(max_waits, len(waits), max_waits):
        nop = nc.sync.nop(nofuse=True, hint="drain_wait_split")
        nop.ins.sync_info = mybir.SyncInfo(
            on_wait=waits[i : i + max_waits], on_update=[]
        )


def _patched_drain_and_barrier(self, tick_clock, wait_clock):
    nc = self.nc
    drain_inst = nc.sync.drain()
    wait_clock.add_sem_waits(
        drain_inst.ins, _ScopedClock({None: tick_clock.global_clock})
    )
    _split_sync_waits(nc, drain_inst)
    nc.all_engine_barrier()
    assert self.sems is not None
    popped = nc._tile_sem_poison_stack.pop()
    assert popped is self._sem_poison
    nc.clear_and_free_semaphores(list(self.sems.allocated().values()))
    nc.all_engine_barrier()


tile.TileContext._drain_and_barrier = _patched_drain_and_barrier


# Same walrus limitation, applied globally: any instruction carrying more
# than one SyncWait gets the extra waits hoisted onto same-engine NoOps
# inserted immediately before it in the BIR json.
import json as _json
import concourse.bass2jax as _bass2jax

_orig_compile_bir_kernel = _bass2jax.compile_bir_kernel


def _split_waits_in_bir(bir_json: bytes) -> bytes:
    j = _json.loads(bir_json)
    n_new = [0]
    for fn in j["functions"]:
        for bb in fn["blocks"]:
            insts = bb["instructions"]
            out = []
            for inst in insts:
                si = inst.get("sync_info")
                waits = (si or {}).get("on_wait") or []
                if len(waits) > 1:
                    for w in waits[:-1]:
                        n_new[0] += 1
                        out.append({
                            "engine": inst["engine"],
                            "ins": [], "outs": [],
                            "name": f"{inst['name']}-ws{n_new[0]}",
                            "opcode": "NoOp",
                            "sync_info": {"on_wait": [w], "on_update": []},
                        })
                    si["on_wait"] = [waits[-1]]
                out.append(inst)
            bb["instructions"] = out
    return _json.dumps(j).encode()


def _patched_compile_bir_kernel(bir_json, tmpdir, neff_name="file.neff"):
    return _orig_compile_bir_kernel(_split_waits_in_bir(bir_json), tmpdir, neff_name)


_bass2jax.compile_bir_kernel = _patched_compile_bir_kernel
# ----------------------------------------------------------------------------

B, N, E, H = 4, 2048, 768, 12
D = 64
RDIM = 32
EPS = 1e-6
XPOS_SCALE_BASE = 512.0
THETA = 10000.0

HL = 6            # heads per core
EL = HL * D       # 384 local embed cols
P = 128
NT = N // P       # 16 row tiles
QC = 512          # q chunk (columns of S^T tiles)
NQC = N // QC     # 4
TW = 2 * (D + RDIM)   # tabs width per tile (k/q cos+sin) = 192
F32 = mybir.dt.float32
BF16 = mybir.dt.bfloat16
I32 = mybir.dt.int32

_CACHE = {}


def _head_perm():
    """Per-head column permutation: rotary evens, rotary odds, passthrough."""
    p = list(range(0, RDIM, 2)) + list(range(1, RDIM, 2)) + list(range(RDIM, D))
    return np.array(p, dtype=np.int64)


def _build_tables(scale_vec, invert_xpos):
    """cosPt [N, 64], sinPt [N, 32] coefficient tables in permuted layout."""
    inv_freq = 1.0 / (THETA ** (np.arange(0, RDIM, 2, dtype=np.float64) / RDIM))
    t = np.arange(N, dtype=np.float64)
    freqs = t[:, None] * inv_freq[None, :]           # [N, 16]
    cos0, sin0 = np.cos(freqs), np.sin(freqs)
    base = (np.arange(0, RDIM, 2, dtype=np.float64) + 0.4 * RDIM) / (1.4 * RDIM)
    power = (t - N // 2) / XPOS_SCALE_BASE
    xsc = base[None, :] ** power[:, None]            # [N, 16]
    if invert_xpos:
        xsc = 1.0 / xsc
    sc = np.asarray(scale_vec, dtype=np.float64)
    cosPt = np.empty((N, D), dtype=np.float64)
    sinPt = np.empty((N, RDIM), dtype=np.float64)
    i = np.arange(16)
    cosPt[:, 0:16] = cos0 * xsc * sc[2 * i][None, :]
    cosPt[:, 16:32] = cos0 * xsc * sc[2 * i + 1][None, :]
    cosPt[:, 32:] = sc[RDIM:][None, :]
    sinPt[:, 0:16] = -sin0 * xsc * sc[2 * i + 1][None, :]
    sinPt[:, 16:32] = sin0 * xsc * sc[2 * i][None, :]
    return cosPt.astype(np.float32), sinPt.astype(np.float32)


def build_graph():
    nc = bass.Bass()
    q_ext = nc.declare_dram_parameter("q", [N, EL], BF16, isOutput=False)
    k_ext = nc.declare_dram_parameter("k", [N, EL], BF16, isOutput=False)
    v_ext = nc.declare_dram_parameter("v", [P, NT * HL * (D + 1)], BF16, isOutput=False)
    wt_ext = nc.declare_dram_parameter("wt", [P, 3 * E], BF16, isOutput=False)
    tabs_ext = nc.declare_dram_parameter(
        "tabs", [P, NT * TW], BF16, isOutput=False)
    tri_ext = nc.declare_dram_parameter("tri", [P, P], BF16, isOutput=False)
    out_ext = nc.declare_dram_parameter("out", [N, E], BF16, isOutput=True)

    q_t4 = q_ext.rearrange("(g t p) e -> g p t e", t=4, p=P)
    k_t4 = k_ext.rearrange("(g t p) e -> g p t e", t=4, p=P)
    out_t = out_ext.rearrange("(t p) e -> t p e", p=P)

    with tile.TileContext(nc) as tc:
        with (
            tc.tile_pool(name="persist", bufs=1) as persist,
            tc.tile_pool(name="qk_in", bufs=3) as qk_in,
            tc.tile_pool(name="pp", bufs=2) as pp,
            tc.tile_pool(name="pp_small", bufs=2) as pp_small,
            tc.tile_pool(name="pt_pool", bufs=22) as pt_pool,
            tc.tile_pool(name="ypre", bufs=3) as ypre_pool,
            tc.tile_pool(name="yt_pool", bufs=3) as yt_pool,
            tc.tile_pool(name="recip", bufs=8) as recip_pool,
            tc.tile_pool(name="outsb", bufs=4) as outsb_pool,
            tc.tile_pool(name="ps_s", bufs=2, space="PSUM") as ps_s,
            tc.tile_pool(name="ps_y", bufs=2, space="PSUM") as ps_y,
            tc.tile_pool(name="ps_t", bufs=1, space="PSUM") as ps_t,
            tc.tile_pool(name="ps_o", bufs=1, space="PSUM") as ps_o,
        ):
            # ---------------- constants (host-prepared layouts) ----------------
            ident = persist.tile([P, P], BF16)
            from concourse.masks import make_identity
            make_identity(nc, ident)
            tabs_sb = persist.tile([P, NT, 2, D + RDIM], BF16)
            tri_sb = persist.tile([P, P], BF16)
            wt_sb = persist.tile([P, 3, E], BF16)
            vall = persist.tile([P, NT, HL, D + 1], BF16)

            # transposed q', k': [128 = 2-head d, hp, n]
            qT = persist.tile([P, 3, N], BF16, name="qT")
            kT = persist.tile([P, 3, N], BF16, name="kT")

            # rotating PSUM transpose slot triples: 2 x [P, 3, 128] bf16
            psT = ps_t.tile([P, 6, P], BF16, name="psT")
            slot_ctr = [0]

            def next_slot3():
                b = (slot_ctr[0] % 2) * 3
                slot_ctr[0] += 1
                return b

            def ap4(t, offset, dims):
                return bass.AP(tensor=t.tensor, offset=t.offset + offset,
                               ap=[t.ap[0]] + dims)

            # ---------------- preprocess helpers ----------------
            def rstd_chain(ssum, nt, tag):
                """ssum [P, nt, 2HL] f32 -> rsqrt(ssum) (DVE, Quake+Newton).

                rsqrt(ssum_k) = 0.125*rstd_k (folds 1/sqrt(D) for scores);
                rsqrt(ssum_q) = 0.125*rstd_q, with the 8x folded into the
                host-side q rope tables.  eps is dropped: ssum ~ chi2(64)
                is bounded well away from 0 for these inputs.
                """
                ish = pp_small.tile([P, nt, 2 * HL], I32, tag=f"i{tag}", name="ish")
                nc.vector.tensor_scalar(out=ish[:], in0=ssum.bitcast(I32),
                                        scalar1=1, scalar2=None,
                                        op0=mybir.AluOpType.logical_shift_right)
                y0i = pp_small.tile([P, nt, 2 * HL], I32, tag=f"y{tag}", name="y0i")
                nc.vector.tensor_scalar(out=y0i[:], in0=ish[:],
                                        scalar1=-1, scalar2=0x5F3759DF,
                                        op0=mybir.AluOpType.mult,
                                        op1=mybir.AluOpType.add)
                y = y0i.bitcast(F32)
                rstd = pp_small.tile([P, nt, 2 * HL], F32, tag=f"r{tag}", name="rstd")
                t_nr = pp_small.tile([P, nt, 2 * HL], F32, tag=f"t{tag}", name="t_nr")
                nc.vector.tensor_mul(t_nr[:], y, y)
                nc.vector.tensor_mul(t_nr[:], t_nr[:], ssum[:])
                nc.vector.tensor_scalar(out=t_nr[:], in0=t_nr[:], scalar1=-0.5,
                                        scalar2=1.5, op0=mybir.AluOpType.mult,
                                        op1=mybir.AluOpType.add)
                nc.vector.tensor_mul(rstd[:], y, t_nr[:])
                return rstd

            def rope_side(xg, t0, nt, i0, side, rstd, rstd_t0,
                          eng_big, eng_small):
                """Rope+norm for group tiles t0..t0+nt of one side.

                Writes a [P, 4, HL, D] tile's [t0:t0+nt] slice and returns it.
                xg [P, 2, 4, HL, D]; tables rows i0..i0+nt; rstd [P, *, 2HL]
                with tile offset rstd_t0.  eng_big runs the two [*,D]-wide
                muls, eng_small the three [*,16/32]-wide rotation ops.
                """
                tag = "kq"[side]
                xh = ap4(xg, side * 4 * HL * D + t0 * HL * D,
                         [[HL * D, nt], [D, HL], [1, D]])
                cosb = ap4(tabs_sb, i0 * TW + side * (TW // 2),
                           [[TW, nt], [0, HL], [1, D]])
                pre = pp.tile([P, 4, HL, D], BF16, tag=f"pre{tag}", name="pre")
                pre_s = ap4(pre, t0 * HL * D, [[HL * D, nt], [D, HL], [1, D]])
                eng_big.tensor_mul(pre_s, xh, cosb)
                tmp = pp.tile([P, 4, HL, RDIM], BF16, tag=f"tmp{tag}", name="tmp")
                xh_odd = ap4(xg, side * 4 * HL * D + t0 * HL * D + 16,
                             [[HL * D, nt], [D, HL], [1, 16]])
                xh_evn = ap4(xg, side * 4 * HL * D + t0 * HL * D,
                             [[HL * D, nt], [D, HL], [1, 16]])
                sin_lo = ap4(tabs_sb, i0 * TW + side * (TW // 2) + D,
                             [[TW, nt], [0, HL], [1, 16]])
                sin_hi = ap4(tabs_sb, i0 * TW + side * (TW // 2) + D + 16,
                             [[TW, nt], [0, HL], [1, 16]])
                tmp_lo = ap4(tmp, t0 * HL * RDIM,
                             [[HL * RDIM, nt], [RDIM, HL], [1, 16]])
                tmp_hi = ap4(tmp, t0 * HL * RDIM + 16,
                             [[HL * RDIM, nt], [RDIM, HL], [1, 16]])
                eng_small.tensor_mul(tmp_lo, xh_odd, sin_lo)
                eng_small.tensor_mul(tmp_hi, xh_evn, sin_hi)
                pre_rot = ap4(pre, t0 * HL * D, [[HL * D, nt], [D, HL], [1, RDIM]])
                tmp_s = ap4(tmp, t0 * HL * RDIM,
                            [[HL * RDIM, nt], [RDIM, HL], [1, RDIM]])
                eng_small.tensor_add(pre_rot, pre_rot, tmp_s)
                a = pp.tile([P, 4, HL, D], BF16, tag=f"a{tag}", name="a")
                a_s = ap4(a, t0 * HL * D, [[HL * D, nt], [D, HL], [1, D]])
                rstd_b = ap4(rstd, rstd_t0 * 2 * HL + side * HL,
                             [[2 * HL, nt], [1, HL], [0, D]])
                eng_big.tensor_mul(a_s, pre_s, rstd_b)
                return a

            def emit_tp(a, t_local, i_global, dstT):
                b = next_slot3()
                for hp in range(3):
                    nc.tensor.transpose(
                        psT[:, b + hp, :],
                        a[:, t_local, 2 * hp: 2 * hp + 2, :], ident[:])
                nc.vector.tensor_copy(
                    dstT[:, :, i_global * P:(i_global + 1) * P],
                    psT[:, b:b + 3, :])

            def preprocess_startup():
                """Group 0 with per-tile DMAs and a tile-0 latency fast path."""
                xg = qk_in.tile([P, 2, 4, HL, D], BF16, tag="xqk", name="x0")
                # tile 0 + its rope tables first (startup critical path),
                # then the remaining tiles
                nc.sync.dma_start(out=xg[:, 0, 0], in_=k_t4[0][:, 0])
                nc.sync.dma_start(out=xg[:, 1, 0], in_=q_t4[0][:, 0])
                nc.sync.dma_start(out=tabs_sb[:, 0:4], in_=tabs_ext[:, 0:4 * TW])
                for ii in range(1, 4):
                    nc.sync.dma_start(out=xg[:, 0, ii], in_=k_t4[0][:, ii])
                    nc.sync.dma_start(out=xg[:, 1, ii], in_=q_t4[0][:, ii])
                nc.sync.dma_start(out=tri_sb[:], in_=tri_ext[:])
                # independent per-DMA-arrival chains: tile 0, tiles 1-2,
                # tile 3 — each tile's transposes flow as its data lands
                def chain(t_lo, nt, tag, ke=nc.gpsimd, qe=nc.vector):
                    sq = pp.tile([P, 2, nt, HL, D], BF16, tag=f"sq{tag}",
                                 name=f"sq{tag}")
                    for s in range(2):
                        nc.scalar.activation(
                            out=sq[:, s], in_=xg[:, s, t_lo:t_lo + nt],
                            func=mybir.ActivationFunctionType.Square)
                    ss = pp_small.tile([P, nt, 2 * HL], F32, tag=f"ss{tag}",
                                       name=f"ss{tag}")
                    nc.vector.reduce_sum(
                        ap4(ss, 0, [[2 * HL, nt], [1, HL]]), sq[:, 0],
                        axis=mybir.AxisListType.X)
                    nc.vector.reduce_sum(
                        ap4(ss, HL, [[2 * HL, nt], [1, HL]]), sq[:, 1],
                        axis=mybir.AxisListType.X)
                    rstd = rstd_chain(ss, nt, tag)
                    ak = rope_side(xg, t_lo, nt, t_lo, 0, rstd, 0, ke, ke)
                    for t in range(nt):
                        emit_tp(ak, t_lo + t, t_lo + t, kT)
                    aq = rope_side(xg, t_lo, nt, t_lo, 1, rstd, 0, qe, qe)
                    for t in range(nt):
                        emit_tp(aq, t_lo + t, t_lo + t, qT)

                chain(0, 1, "0")
                chain(1, 2, "12")
                chain(3, 1, "3", ke=nc.vector, qe=nc.gpsimd)

            def preprocess_group(i0):
                """Batched 4-tile group load + norm stats (GpSimd bulk)."""
                g = i0 // 4
                xg = qk_in.tile([P, 2, 4, HL, D], BF16, tag="xqk", name="x")
                nc.sync.dma_start(out=xg[:, 0], in_=k_t4[g])
                nc.sync.dma_start(out=xg[:, 1], in_=q_t4[g])
                sq = pp.tile([P, 2, 4, HL, D], BF16, tag="sq", name="sq")
                for s in range(2):
                    nc.scalar.activation(
                        out=sq[:, s], in_=xg[:, s],
                        func=mybir.ActivationFunctionType.Square)
                ssum = pp_small.tile([P, 4, 2 * HL], F32, tag="ss4", name="ssum")
                nc.vector.reduce_sum(
                    ap4(ssum, 0, [[2 * HL, 4], [1, HL]]), sq[:, 0],
                    axis=mybir.AxisListType.X)
                nc.vector.reduce_sum(
                    ap4(ssum, HL, [[2 * HL, 4], [1, HL]]), sq[:, 1],
                    axis=mybir.AxisListType.X)
                rstd = rstd_chain(ssum, 4, "4")
                return xg, rstd

            def finish_group_side(grp, i0, side):
                """Rope+norm+transpose all 4 tiles of one side (GpSimd bulk)."""
                xg, rstd = grp
                a = rope_side(xg, 0, 4, i0, side, rstd, 0,
                              nc.gpsimd, nc.gpsimd)
                dstT = (kT, qT)[side]
                for t in range(4):
                    emit_tp(a, t, i0 + t, dstT)

            # ---------------- attention emit helpers ----------------
            def emit_qk_exp(h, qc, filler, split_q=False):
                """Scores + exp + diag mask for (head, q-chunk). Returns pt list."""
                hp, hh = divmod(h, 2)
                hoff = 64 * hh
                nkt = 4 * (qc + 1)
                ngroups = nkt // 2
                pts = []
                for kg in range(ngroups):
                    ss = ps_s.tile([P, 2, QC], F32, tag="ps_s", name="ss")
                    for j in range(2):
                        kt = kg * 2 + j
                        doff = kt - 4 * qc
                        qstart = max(doff, 0) * P
                        if split_q and kg < 2:
                            # per-q-tile matmuls so the first scores start as
                            # soon as each transposed tile lands (startup)
                            for qt in range(qstart // P, 4):
                                diag = qt == doff
                                nc.tensor.matmul(
                                    ss[:, j, qt * P:(qt + 1) * P],
                                    kT[hoff:hoff + 64, hp, kt * P:(kt + 1) * P],
                                    qT[hoff:hoff + 64, hp,
                                       qc * QC + qt * P: qc * QC + (qt + 1) * P],
                                    start=True, stop=not diag,
                                )
                                if diag:
                                    # causal mask: -1e30 bias closes the group
                                    nc.tensor.matmul(
                                        ss[:, j, qt * P:(qt + 1) * P],
                                        ident[:], tri_sb[:],
                                        start=False, stop=True,
                                    )
                            continue
                        nc.tensor.matmul(
                            ss[:, j, qstart:QC],
                            kT[hoff : hoff + 64, hp, kt * P : (kt + 1) * P],
                            qT[hoff : hoff + 64, hp, qc * QC + qstart : (qc + 1) * QC],
                            start=True, stop=doff < 0,
                        )
                        if doff >= 0:
                            # causal mask as -1e30 bias on the diagonal block,
                            # same accumulation group as the score matmul (no
                            # DVE dependency on the exp->AV path)
                            nc.tensor.matmul(
                                ss[:, j, doff * P:(doff + 1) * P],
                                ident[:], tri_sb[:],
                                start=False, stop=True,
                            )
                    pt = pt_pool.tile([P, 2, QC], BF16, tag="pt", name="pt")
                    # exp only the causally-live q range (union over the pair)
                    qmin = max(kg * 2 - 4 * qc, 0) * P
                    nc.scalar.activation(
                        out=pt[:, :, qmin:], in_=ss[:, :, qmin:],
                        func=mybir.ActivationFunctionType.Exp,
                    )
                    pts.append(pt)
                    # interleave prev head's AV units as PE filler
                    if filler:
                        n = -(-len(filler) // (ngroups - kg))
                        for _ in range(min(n, len(filler))):
                            filler.pop(0)()
                return pts

            def make_av_units(h, qc, pts, ypre3):
                """Per-kt AV emission closures + final normalize closure."""
                hp, hh = divmod(h, 2)
                nkt = 4 * (qc + 1)
                ys = ps_y.tile([P, 4, D + 1], F32, tag="ps_y", name="ys")

                def mk(kt):
                    def unit():
                        pt = pts[kt // 2]
                        j = kt % 2
                        for qs in range(4):
                            first = kt == 0 and qs == 0
                            last = kt == nkt - 1 and qs == 3
                            if (not first and not last
                                    and kt * P >= qc * QC + (qs + 1) * P):
                                continue
                            nc.tensor.matmul(
                                ys[:, qs, :],
                                pt[:, j, qs * P : (qs + 1) * P],
                                vall[:, kt, h, :],
                                start=first, stop=last,
                            )
                    return unit

                def fin():
                    r = recip_pool.tile([P, 4], F32, tag="recip", name="r")
                    nc.vector.reciprocal(out=r[:], in_=ys[:, :, D])
                    r_b = bass.AP(tensor=r.tensor, offset=r.offset,
                                  ap=[r.ap[0], r.ap[1], [0, D]])
                    nc.vector.tensor_mul(
                        ypre3[hp][:, :, hh, :], ys[:, :, 0:D], r_b)

                return [mk(kt) for kt in range(nkt)] + [fin]

            def proj_subunits(qc, ypre3, qs):
                """One row tile's projection as 4 filler closures."""
                nt_i = qc * 4 + qs
                box = {}

                def t_unit():
                    box["yt"] = yt_pool.tile([P, 3, P], BF16, tag="yt",
                                             name="yt")
                    box["osb"] = outsb_pool.tile([P, E], BF16, tag="osb",
                                                 name="osb")
                    box["po"] = ps_o.tile([P, 2, 256], F32, tag="ps_o",
                                          name="po")
                    b = next_slot3()
                    for hp in range(3):
                        nc.tensor.transpose(
                            psT[:, b + hp, :], ypre3[hp][:, qs, :, :], ident[:]
                        )
                    nc.vector.tensor_copy(box["yt"][:, :, :], psT[:, b:b + 3, :])

                def o_unit(oh):
                    def u():
                        yt, osb, po = box["yt"], box["osb"], box["po"]
                        sl = oh % 2
                        for ec in range(3):
                            nc.tensor.matmul(
                                po[:, sl, :],
                                yt[:, ec, :],
                                wt_sb[:, ec, oh * 256 : (oh + 1) * 256],
                                start=(ec == 0), stop=(ec == 2),
                            )
                        nc.vector.tensor_copy(
                            osb[:, oh * 256 : (oh + 1) * 256], po[:, sl, :]
                        )
                        if oh == 2:
                            nc.sync.dma_start(out=out_t[nt_i], in_=osb[:])
                    return u

                return [t_unit, o_unit(0), o_unit(1), o_unit(2)]

            # ---------------- main pipelined schedule ----------------
            pending_av = []     # at most one (h, qc, pts, ypre3)
            pending_proj = []   # (qc, ypre3, qs) row-tile units
            preprocess_startup()
            for qc in range(NQC):
                ypre3_cur = [
                    ypre_pool.tile([P, 4, 2, D], BF16, tag=f"ypre{_i}",
                                   name=f"ypre{_i}")
                    for _i in range(3)
                ]
                for h in range(HL):
                    if qc == 0 and h == 1:
                        # bulk loads after the startup-critical q/k DMAs:
                        # v first (first AV filler), then the remaining rope
                        # tables (group-1 finish), then weights (projection)
                        nc.sync.dma_start(out=vall[:], in_=v_ext[:])
                        nc.sync.dma_start(out=tabs_sb[:, 4:NT],
                                          in_=tabs_ext[:, 4 * TW:])
                    filler = []
                    meta = None
                    if pending_av:
                        ph, pqc, ppts, pypre = pending_av.pop()
                        filler = make_av_units(ph, pqc, ppts, pypre)
                        meta = (ph, pqc, pypre)
                    for _ in range(5):
                        if pending_proj:
                            filler.append(pending_proj.pop(0))
                    pts = emit_qk_exp(h, qc, filler, split_q=(h == 0))
                    for u in filler:
                        u()
                    if meta is not None:
                        ph, pqc, pypre = meta
                        if ph == HL - 1:
                            for qs in range(4):
                                pending_proj.extend(
                                    proj_subunits(pqc, pypre, qs))
                    pending_av.append((h, qc, pts, ypre3_cur))
                    # preprocess next q-chunk's tiles inside the head loop,
                    # k/q sides split across head windows to avoid bursts
                    # (qc0 starts at h==1 to keep startup uncongested)
                    if qc < NQC - 1:
                        hofs = 1 if qc == 0 else 0
                        if h == hofs:
                            grp = preprocess_group(4 * (qc + 1))
                        elif h == hofs + 1:
                            if qc == 0:
                                nc.sync.dma_start(out=wt_sb[:], in_=wt_ext[:])
                            finish_group_side(grp, 4 * (qc + 1), 0)
                        elif h == hofs + 2:
                            finish_group_side(grp, 4 * (qc + 1), 1)
            # drain
            ph, pqc, ppts, pypre = pending_av.pop()
            for u in make_av_units(ph, pqc, ppts, pypre):
                u()
            while pending_proj:
                pending_proj.pop(0)()
            for qs in range(4):
                for u in proj_subunits(pqc, pypre, qs):
                    u()
    return nc


def _get_graph():
    if "nc" not in _CACHE:
        _CACHE["nc"] = build_graph()
    return _CACHE["nc"]


def _host_inputs(q, k, v, q_scale, k_scale, proj_w):
    perm = _head_perm()
    bf = ml_dtypes.bfloat16
    qcos, qsin = _build_tables(q_scale, invert_xpos=False)
    kcos, ksin = _build_tables(k_scale, invert_xpos=True)
    # device computes rsqrt(ssum_q) = 0.125*rstd_q; fold the 8x here
    qcos, qsin = qcos * 8.0, qsin * 8.0

    def tab_layout(t):
        # [N, w] -> [128, NT*w] matching sbuf tile [P, NT, w]
        w = t.shape[1]
        return np.ascontiguousarray(
            t.reshape(NT, P, w).transpose(1, 0, 2).reshape(P, NT * w)).astype(bf)

    qcos_r, qsin_r = tab_layout(qcos), tab_layout(qsin)
    kcos_r, ksin_r = tab_layout(kcos), tab_layout(ksin)
    # combined [P, NT, 2, D+RDIM]: slot 0 = k tables, slot 1 = q tables
    tabs = np.empty((P, NT, 2, D + RDIM), dtype=kcos_r.dtype)
    tabs[:, :, 0, 0:D] = kcos_r.reshape(P, NT, D)
    tabs[:, :, 0, D:] = ksin_r.reshape(P, NT, RDIM)
    tabs[:, :, 1, 0:D] = qcos_r.reshape(P, NT, D)
    tabs[:, :, 1, D:] = qsin_r.reshape(P, NT, RDIM)
    tabs_r = np.ascontiguousarray(tabs.reshape(P, NT * TW))
    # additive causal mask for the diagonal [128,128] sub-block:
    # tri[k, q] = 0 where q >= k else -1e30 (exp -> exactly 0)
    tri = np.where(np.arange(P)[None, :] >= np.arange(P)[:, None],
                   0.0, -1e30).astype(np.float32)
    tri_r = np.ascontiguousarray(tri).astype(bf)

    in_maps = []
    for c in range(8):
        b = c // 2
        h0 = HL * (c % 2)
        cols = np.concatenate([(h0 + h) * D + perm for h in range(HL)])
        vcols = np.arange(h0 * D, (h0 + HL) * D)
        v_aug = np.ones((N, HL, D + 1), np.float32)
        v_aug[:, :, :D] = v[b][:, vcols].reshape(N, HL, D)
        wt_l = np.ascontiguousarray(proj_w[:, vcols].T)   # [384, 768]
        wt_r = np.ascontiguousarray(
            wt_l.reshape(3, P, E).transpose(1, 0, 2).reshape(P, 3 * E))
        in_maps.append({
            "q": np.ascontiguousarray(q[b][:, cols]).astype(bf),
            "k": np.ascontiguousarray(k[b][:, cols]).astype(bf),
            "v": np.ascontiguousarray(
                v_aug.reshape(NT, P, HL * (D + 1)).transpose(1, 0, 2)
                .reshape(P, NT * HL * (D + 1))).astype(bf),
            "wt": wt_r.astype(bf),
            "tabs": tabs_r,
            "tri": tri_r,
        })
    return in_maps


def kernel(q, k, v, q_scale, k_scale, proj_w, proj_b):
    nc = _get_graph()
    in_maps = _host_inputs(q, k, v, q_scale, k_scale, proj_w)
    res = run_bass_kernel_spmd(nc, in_maps, list(range(8)))
    out = np.empty((B, N, E), np.float32)
    for b in range(B):
        out[b] = (res.results[2 * b]["out"].astype(np.float32)
                  + res.results[2 * b + 1]["out"].astype(np.float32))
    out += proj_b[None, None, :].astype(np.float32)
    return out


# revision 28
# speedup vs baseline: 1.0314x; 1.0083x over previous
"""Distributed Trainium2 kernel for nn_Attention_64742337020012.

B=4, N=2048, E=768, H=12, D=64 causal attention with per-head RMS norm,
interleaved xpos RoPE, and output projection.

Sharding: 8 cores, core c owns batch c//2 and heads 6*(c%2) .. 6*(c%2)+6
(head-independent attention).  Each core computes full causal attention for
its 6 heads over all 2048 positions plus the partial output projection using
its 384 rows of proj_w^T; the host sums the two partial projections per batch
and adds the bias.

Pipeline (all matmuls bf16, f32 accumulation):
  1. q,k loaded bf16 (cast on host), roped via host-precomputed coefficient
     tables (head dim pre-permuted evens-first); rsqrt(ms) via Quake seed +
     1 Newton step on DVE batched across 4-tile groups.  Rope/norm bulk ops
     are 4-tile fused on GpSimd; latency-critical small ops stay on DVE.
  2. q',k' transposed to [d, n] via PE transposes into 2 rotating PSUM slot
     TRIPLES; each triple drains with a single [P,3,128] DVE copy.
  3. Scores computed transposed (S^T[k,q] tiles), causal-trimmed, exp on ACT
     straight out of PSUM, paired triangular-mask multiplies on DVE.
  4. AV with P^T stationary and [V | 1] moving gives y and the softmax
     denominator in one accumulation; per-row reciprocal normalizes.
     PE stream software-pipelined one head deep with AV/proj filler.
  5. y transposed on PE, projected in 3x256-col chunks, bf16 partials
     written to DRAM (host sums the two per-batch partials in f32).
Startup is latency-tuned: per-tile q/k group-0 DMAs ahead of the rope
tables (split so tiles 0-3's tables land first), tile-0 norm fast path,
bulk v/wt/tabs loads deferred past the startup-critical DMAs.
"""

import sys

sys.path.insert(0, "/opt/trn_rl_repo")

import numpy as np
import ml_dtypes

import concourse.bass as bass
import concourse.mybir as mybir
import concourse.tile as tile
from concourse.bass_utils import run_bass_kernel_spmd

# ----------------------------------------------------------------------------
# Workaround for this container's walrus build: the TileContext tail drain
# carries one SyncWait per outstanding semaphore, but CoreV3 CTRL codegen
# accepts only a single sync wait per instruction.  Split the waits across
# single-wait NOPs emitted right after the drain.
from concourse.vector_clock import ScopedClock as _ScopedClock


def _split_sync_waits(nc, inst, max_waits=1):
    si = inst.ins.sync_info
    if si is None:
        return
    waits = list(si.on_wait)
    if len(waits) <= max_waits:
        return
    inst.ins.sync_info = mybir.SyncInfo(
        on_wait=waits[:max_waits], on_update=list(si.on_update)
    )
    for i in range(max_waits, len(waits), max_waits):
        nop = nc.sync.nop(nofuse=True, hint="drain_wait_split")
        nop.ins.sync_info = mybir.SyncInfo(
            on_wait=waits[i : i + max_waits], on_update=[]
        )


def _patched_drain_and_barrier(self, tick_clock, wait_clock):
    nc = self.nc
    drain_inst = nc.sync.drain()
    wait_clock.add_sem_waits(
        drain_inst.ins, _ScopedClock({None: tick_clock.global_clock})
    )
    _split_sync_waits(nc, drain_inst)
    nc.all_engine_barrier()
    assert self.sems is not None
    popped = nc._tile_sem_poison_stack.pop()
    assert popped is self._sem_poison
    nc.clear_and_free_semaphores(list(self.sems.allocated().values()))
    nc.all_engine_barrier()


tile.TileContext._drain_and_barrier = _patched_drain_and_barrier


# Same walrus limitation, applied globally: any instruction carrying more
# than one SyncWait gets the extra waits hoisted onto same-engine NoOps
# inserted immediately before it in the BIR json.
import json as _json
import concourse.bass2jax as _bass2jax

_orig_compile_bir_kernel = _bass2jax.compile_bir_kernel


def _split_waits_in_bir(bir_json: bytes) -> bytes:
    j = _json.loads(bir_json)
    n_new = [0]
    for fn in j["functions"]:
        for bb in fn["blocks"]:
            insts = bb["instructions"]
            out = []
            for inst in insts:
                si = inst.get("sync_info")
                waits = (si or {}).get("on_wait") or []
                if len(waits) > 1:
                    for w in waits[:-1]:
                        n_new[0] += 1
                        out.append({
                            "engine": inst["engine"],
                            "ins": [], "outs": [],
                            "name": f"{inst['name']}-ws{n_new[0]}",
                            "opcode": "NoOp",
                            "sync_info": {"on_wait": [w], "on_update": []},
                        })
                    si["on_wait"] = [waits[-1]]
                out.append(inst)
            bb["instructions"] = out
    return _json.dumps(j).encode()


def _patched_compile_bir_kernel(bir_json, tmpdir, neff_name="file.neff"):
    return _orig_compile_bir_kernel(_split_waits_in_bir(bir_json), tmpdir, neff_name)


_bass2jax.compile_bir_kernel = _patched_compile_bir_kernel
# ----------------------------------------------------------------------------

B, N, E, H = 4, 2048, 768, 12
D = 64
RDIM = 32
EPS = 1e-6
XPOS_SCALE_BASE = 512.0
THETA = 10000.0

HL = 6            # heads per core
EL = HL * D       # 384 local embed cols
P = 128
NT = N // P       # 16 row tiles
QC = 512          # q chunk (columns of S^T tiles)
NQC = N // QC     # 4
TW = 2 * (D + RDIM)   # tabs width per tile (k/q cos+sin) = 192
F32 = mybir.dt.float32
BF16 = mybir.dt.bfloat16
I32 = mybir.dt.int32

_CACHE = {}


def _head_perm():
    """Per-head column permutation: rotary evens, rotary odds, passthrough."""
    p = list(range(0, RDIM, 2)) + list(range(1, RDIM, 2)) + list(range(RDIM, D))
    return np.array(p, dtype=np.int64)


def _build_tables(scale_vec, invert_xpos):
    """cosPt [N, 64], sinPt [N, 32] coefficient tables in permuted layout."""
    inv_freq = 1.0 / (THETA ** (np.arange(0, RDIM, 2, dtype=np.float64) / RDIM))
    t = np.arange(N, dtype=np.float64)
    freqs = t[:, None] * inv_freq[None, :]           # [N, 16]
    cos0, sin0 = np.cos(freqs), np.sin(freqs)
    base = (np.arange(0, RDIM, 2, dtype=np.float64) + 0.4 * RDIM) / (1.4 * RDIM)
    power = (t - N // 2) / XPOS_SCALE_BASE
    xsc = base[None, :] ** power[:, None]            # [N, 16]
    if invert_xpos:
        xsc = 1.0 / xsc
    sc = np.asarray(scale_vec, dtype=np.float64)
    cosPt = np.empty((N, D), dtype=np.float64)
    sinPt = np.empty((N, RDIM), dtype=np.float64)
    i = np.arange(16)
    cosPt[:, 0:16] = cos0 * xsc * sc[2 * i][None, :]
    cosPt[:, 16:32] = cos0 * xsc * sc[2 * i + 1][None, :]
    cosPt[:, 32:] = sc[RDIM:][None, :]
    sinPt[:, 0:16] = -sin0 * xsc * sc[2 * i + 1][None, :]
    sinPt[:, 16:32] = sin0 * xsc * sc[2 * i][None, :]
    return cosPt.astype(np.float32), sinPt.astype(np.float32)


def build_graph():
    nc = bass.Bass()
    q_ext = nc.declare_dram_parameter("q", [N, EL], BF16, isOutput=False)
    k_ext = nc.declare_dram_parameter("k", [N, EL], BF16, isOutput=False)
    v_ext = nc.declare_dram_parameter("v", [P, NT * HL * (D + 1)], BF16, isOutput=False)
    wt_ext = nc.declare_dram_parameter("wt", [P, 3 * E], BF16, isOutput=False)
    tabs_ext = nc.declare_dram_parameter(
        "tabs", [P, NT * TW], BF16, isOutput=False)
    tri_ext = nc.declare_dram_parameter("tri", [P, P], BF16, isOutput=False)
    out_ext = nc.declare_dram_parameter("out", [N, E], BF16, isOutput=True)

    q_t4 = q_ext.rearrange("(g t p) e -> g p t e", t=4, p=P)
    k_t4 = k_ext.rearrange("(g t p) e -> g p t e", t=4, p=P)
    out_t = out_ext.rearrange("(t p) e -> t p e", p=P)

    with tile.TileContext(nc) as tc:
        with (
            tc.tile_pool(name="persist", bufs=1) as persist,
            tc.tile_pool(name="qk_in", bufs=3) as qk_in,
            tc.tile_pool(name="pp", bufs=2) as pp,
            tc.tile_pool(name="pp_small", bufs=2) as pp_small,
            tc.tile_pool(name="pt_pool", bufs=22) as pt_pool,
            tc.tile_pool(name="ypre", bufs=3) as ypre_pool,
            tc.tile_pool(name="yt_pool", bufs=3) as yt_pool,
            tc.tile_pool(name="recip", bufs=8) as recip_pool,
            tc.tile_pool(name="outsb", bufs=4) as outsb_pool,
            tc.tile_pool(name="ps_s", bufs=2, space="PSUM") as ps_s,
            tc.tile_pool(name="ps_y", bufs=2, space="PSUM") as ps_y,
            tc.tile_pool(name="ps_t", bufs=1, space="PSUM") as ps_t,
            tc.tile_pool(name="ps_o", bufs=1, space="PSUM") as ps_o,
        ):
            # ---------------- constants (host-prepared layouts) ----------------
            ident = persist.tile([P, P], BF16)
            from concourse.masks import make_identity
            make_identity(nc, ident)
            tabs_sb = persist.tile([P, NT, 2, D + RDIM], BF16)
            tri_sb = persist.tile([P, P], BF16)
            wt_sb = persist.tile([P, 3, E], BF16)
            vall = persist.tile([P, NT, HL, D + 1], BF16)

            # transposed q', k': [128 = 2-head d, hp, n]
            qT = persist.tile([P, 3, N], BF16, name="qT")
            kT = persist.tile([P, 3, N], BF16, name="kT")

            # rotating PSUM transpose slot triples: 2 x [P, 3, 128] bf16
            psT = ps_t.tile([P, 6, P], BF16, name="psT")
            slot_ctr = [0]

            def next_slot3():
                b = (slot_ctr[0] % 2) * 3
                slot_ctr[0] += 1
                return b

            def ap4(t, offset, dims):
                return bass.AP(tensor=t.tensor, offset=t.offset + offset,
                               ap=[t.ap[0]] + dims)

            # ---------------- preprocess helpers ----------------
            def rstd_chain(ssum, nt, tag):
                """ssum [P, nt, 2HL] f32 -> rsqrt(ssum) (DVE, Quake+Newton).

                rsqrt(ssum_k) = 0.125*rstd_k (folds 1/sqrt(D) for scores);
                rsqrt(ssum_q) = 0.125*rstd_q, with the 8x folded into the
                host-side q rope tables.  eps is dropped: ssum ~ chi2(64)
                is bounded well away from 0 for these inputs.
                """
                ish = pp_small.tile([P, nt, 2 * HL], I32, tag=f"i{tag}", name="ish")
                nc.vector.tensor_scalar(out=ish[:], in0=ssum.bitcast(I32),
                                        scalar1=1, scalar2=None,
                                        op0=mybir.AluOpType.logical_shift_right)
                y0i = pp_small.tile([P, nt, 2 * HL], I32, tag=f"y{tag}", name="y0i")
                nc.vector.tensor_scalar(out=y0i[:], in0=ish[:],
                                        scalar1=-1, scalar2=0x5F3759DF,
                                        op0=mybir.AluOpType.mult,
                                        op1=mybir.AluOpType.add)
                y = y0i.bitcast(F32)
                rstd = pp_small.tile([P, nt, 2 * HL], F32, tag=f"r{tag}", name="rstd")
                t_nr = pp_small.tile([P, nt, 2 * HL], F32, tag=f"t{tag}", name="t_nr")
                nc.vector.tensor_mul(t_nr[:], y, y)
                nc.vector.tensor_mul(t_nr[:], t_nr[:], ssum[:])
                nc.vector.tensor_scalar(out=t_nr[:], in0=t_nr[:], scalar1=-0.5,
                                        scalar2=1.5, op0=mybir.AluOpType.mult,
                                        op1=mybir.AluOpType.add)
                nc.vector.tensor_mul(rstd[:], y, t_nr[:])
                return rstd

            def rope_side(xg, t0, nt, i0, side, rstd, rstd_t0,
                          eng_big, eng_small):
                """Rope+norm for group tiles t0..t0+nt of one side.

                Writes a [P, 4, HL, D] tile's [t0:t0+nt] slice and returns it.
                xg [P, 2, 4, HL, D]; tables rows i0..i0+nt; rstd [P, *, 2HL]
                with tile offset rstd_t0.  eng_big runs the two [*,D]-wide
                muls, eng_small the three [*,16/32]-wide rotation ops.
                """
                tag = "kq"[side]
                xh = ap4(xg, side * 4 * HL * D + t0 * HL * D,
                         [[HL * D, nt], [D, HL], [1, D]])
                cosb = ap4(tabs_sb, i0 * TW + side * (TW // 2),
                           [[TW, nt], [0, HL], [1, D]])
                pre = pp.tile([P, 4, HL, D], BF16, tag=f"pre{tag}", name="pre")
                pre_s = ap4(pre, t0 * HL * D, [[HL * D, nt], [D, HL], [1, D]])
                eng_big.tensor_mul(pre_s, xh, cosb)
                tmp = pp.tile([P, 4, HL, RDIM], BF16, tag=f"tmp{tag}", name="tmp")
                xh_odd = ap4(xg, side * 4 * HL * D + t0 * HL * D + 16,
                             [[HL * D, nt], [D, HL], [1, 16]])
                xh_evn = ap4(xg, side * 4 * HL * D + t0 * HL * D,
                             [[HL * D, nt], [D, HL], [1, 16]])
                sin_lo = ap4(tabs_sb, i0 * TW + side * (TW // 2) + D,
                             [[TW, nt], [0, HL], [1, 16]])
                sin_hi = ap4(tabs_sb, i0 * TW + side * (TW // 2) + D + 16,
                             [[TW, nt], [0, HL], [1, 16]])
                tmp_lo = ap4(tmp, t0 * HL * RDIM,
                             [[HL * RDIM, nt], [RDIM, HL], [1, 16]])
                tmp_hi = ap4(tmp, t0 * HL * RDIM + 16,
                             [[HL * RDIM, nt], [RDIM, HL], [1, 16]])
                eng_small.tensor_mul(tmp_lo, xh_odd, sin_lo)
                eng_small.tensor_mul(tmp_hi, xh_evn, sin_hi)
                pre_rot = ap4(pre, t0 * HL * D, [[HL * D, nt], [D, HL], [1, RDIM]])
                tmp_s = ap4(tmp, t0 * HL * RDIM,
                            [[HL * RDIM, nt], [RDIM, HL], [1, RDIM]])
                eng_small.tensor_add(pre_rot, pre_rot, tmp_s)
                a = pp.tile([P, 4, HL, D], BF16, tag=f"a{tag}", name="a")
                a_s = ap4(a, t0 * HL * D, [[HL * D, nt], [D, HL], [1, D]])
                rstd_b = ap4(rstd, rstd_t0 * 2 * HL + side * HL,
                             [[2 * HL, nt], [1, HL], [0, D]])
                eng_big.tensor_mul(a_s, pre_s, rstd_b)
                return a

            def emit_tp(a, t_local, i_global, dstT):
                b = next_slot3()
                for hp in range(3):
                    nc.tensor.transpose(
                        psT[:, b + hp, :],
                        a[:, t_local, 2 * hp: 2 * hp + 2, :], ident[:])
                nc.vector.tensor_copy(
                    dstT[:, :, i_global * P:(i_global + 1) * P],
                    psT[:, b:b + 3, :])

            def preprocess_startup():
                """Group 0 with per-tile DMAs and a tile-0 latency fast path."""
                xg = qk_in.tile([P, 2, 4, HL, D], BF16, tag="xqk", name="x0")
                # tile 0 + its rope tables first (startup critical path),
                # then the remaining tiles
                nc.sync.dma_start(out=xg[:, 0, 0], in_=k_t4[0][:, 0])
                nc.sync.dma_start(out=xg[:, 1, 0], in_=q_t4[0][:, 0])
                nc.sync.dma_start(out=tabs_sb[:, 0:4], in_=tabs_ext[:, 0:4 * TW])
                for ii in range(1, 4):
                    nc.sync.dma_start(out=xg[:, 0, ii], in_=k_t4[0][:, ii])
                    nc.sync.dma_start(out=xg[:, 1, ii], in_=q_t4[0][:, ii])
                nc.sync.dma_start(out=tri_sb[:], in_=tri_ext[:])
                # independent per-DMA-arrival chains: tile 0, tiles 1-2,
                # tile 3 — each tile's transposes flow as its data lands
                def chain(t_lo, nt, tag, ke=nc.gpsimd, qe=nc.vector):
                    sq = pp.tile([P, 2, nt, HL, D], BF16, tag=f"sq{tag}",
                                 name=f"sq{tag}")
                    for s in range(2):
                        nc.scalar.activation(
                            out=sq[:, s], in_=xg[:, s, t_lo:t_lo + nt],
                            func=mybir.ActivationFunctionType.Square)
                    ss = pp_small.tile([P, nt, 2 * HL], F32, tag=f"ss{tag}",
                                       name=f"ss{tag}")
                    nc.vector.reduce_sum(
                        ap4(ss, 0, [[2 * HL, nt], [1, HL]]), sq[:, 0],
                        axis=mybir.AxisListType.X)
                    nc.vector.reduce_sum(
                        ap4(ss, HL, [[2 * HL, nt], [1, HL]]), sq[:, 1],
                        axis=mybir.AxisListType.X)
                    rstd = rstd_chain(ss, nt, tag)
                    ak = rope_side(xg, t_lo, nt, t_lo, 0, rstd, 0, ke, ke)
                    for t in range(nt):
                        emit_tp(ak, t_lo + t, t_lo + t, kT)
                    aq = rope_side(xg, t_lo, nt, t_lo, 1, rstd, 0, qe, qe)
                    for t in range(nt):
                        emit_tp(aq, t_lo + t, t_lo + t, qT)

                chain(0, 1, "0")
                chain(1, 2, "12")
                chain(3, 1, "3")

            def preprocess_group(i0):
                """Batched 4-tile group load + norm stats (GpSimd bulk)."""
                g = i0 // 4
                xg = qk_in.tile([P, 2, 4, HL, D], BF16, tag="xqk", name="x")
                nc.sync.dma_start(out=xg[:, 0], in_=k_t4[g])
                nc.sync.dma_start(out=xg[:, 1], in_=q_t4[g])
                sq = pp.tile([P, 2, 4, HL, D], BF16, tag="sq", name="sq")
                for s in range(2):
                    nc.scalar.activation(
                        out=sq[:, s], in_=xg[:, s],
                        func=mybir.ActivationFunctionType.Square)
                ssum = pp_small.tile([P, 4, 2 * HL], F32, tag="ss4", name="ssum")
                nc.vector.reduce_sum(
                    ap4(ssum, 0, [[2 * HL, 4], [1, HL]]), sq[:, 0],
                    axis=mybir.AxisListType.X)
                nc.vector.reduce_sum(
                    ap4(ssum, HL, [[2 * HL, 4], [1, HL]]), sq[:, 1],
                    axis=mybir.AxisListType.X)
                rstd = rstd_chain(ssum, 4, "4")
                return xg, rstd

            def finish_group_side(grp, i0, side):
                """Rope+norm+transpose all 4 tiles of one side (GpSimd bulk)."""
                xg, rstd = grp
                a = rope_side(xg, 0, 4, i0, side, rstd, 0,
                              nc.gpsimd, nc.gpsimd)
                dstT = (kT, qT)[side]
                for t in range(4):
                    emit_tp(a, t, i0 + t, dstT)

            # ---------------- attention emit helpers ----------------
            def emit_qk_exp(h, qc, filler, split_q=False):
                """Scores + exp + diag mask for (head, q-chunk). Returns pt list."""
                hp, hh = divmod(h, 2)
                hoff = 64 * hh
                nkt = 4 * (qc + 1)
                ngroups = nkt // 2
                pts = []
                for kg in range(ngroups):
                    ss = ps_s.tile([P, 2, QC], F32, tag="ps_s", name="ss")
                    for j in range(2):
                        kt = kg * 2 + j
                        doff = kt - 4 * qc
                        qstart = max(doff, 0) * P
                        if split_q and kg < 2:
                            # per-q-tile matmuls so the first scores start as
                            # soon as each transposed tile lands (startup)
                            for qt in range(qstart // P, 4):
                                diag = qt == doff
                                nc.tensor.matmul(
                                    ss[:, j, qt * P:(qt + 1) * P],
                                    kT[hoff:hoff + 64, hp, kt * P:(kt + 1) * P],
                                    qT[hoff:hoff + 64, hp,
                                       qc * QC + qt * P: qc * QC + (qt + 1) * P],
                                    start=True, stop=not diag,
                                )
                                if diag:
                                    # causal mask: -1e30 bias closes the group
                                    nc.tensor.matmul(
                                        ss[:, j, qt * P:(qt + 1) * P],
                                        ident[:], tri_sb[:],
                                        start=False, stop=True,
                                    )
                            continue
                        nc.tensor.matmul(
                            ss[:, j, qstart:QC],
                            kT[hoff : hoff + 64, hp, kt * P : (kt + 1) * P],
                            qT[hoff : hoff + 64, hp, qc * QC + qstart : (qc + 1) * QC],
                            start=True, stop=doff < 0,
                        )
                        if doff >= 0:
                            # causal mask as -1e30 bias on the diagonal block,
                            # same accumulation group as the score matmul (no
                            # DVE dependency on the exp->AV path)
                            nc.tensor.matmul(
                                ss[:, j, doff * P:(doff + 1) * P],
                                ident[:], tri_sb[:],
                                start=False, stop=True,
                            )
                    pt = pt_pool.tile([P, 2, QC], BF16, tag="pt", name="pt")
                    # exp only the causally-live q range (union over the pair)
                    qmin = max(kg * 2 - 4 * qc, 0) * P
                    nc.scalar.activation(
                        out=pt[:, :, qmin:], in_=ss[:, :, qmin:],
                        func=mybir.ActivationFunctionType.Exp,
                    )
                    pts.append(pt)
                    # interleave prev head's AV units as PE filler
                    if filler:
                        n = -(-len(filler) // (ngroups - kg))
                        for _ in range(min(n, len(filler))):
                            filler.pop(0)()
                return pts

            def make_av_units(h, qc, pts, ypre3):
                """Per-kt AV emission closures + final normalize closure."""
                hp, hh = divmod(h, 2)
                nkt = 4 * (qc + 1)
                ys = ps_y.tile([P, 4, D + 1], F32, tag="ps_y", name="ys")

                def mk(kt):
                    def unit():
                        pt = pts[kt // 2]
                        j = kt % 2
                        for qs in range(4):
                            first = kt == 0 and qs == 0
                            last = kt == nkt - 1 and qs == 3
                            if (not first and not last
                                    and kt * P >= qc * QC + (qs + 1) * P):
                                continue
                            nc.tensor.matmul(
                                ys[:, qs, :],
                                pt[:, j, qs * P : (qs + 1) * P],
                                vall[:, kt, h, :],
                                start=first, stop=last,
                            )
                    return unit

                def fin():
                    r = recip_pool.tile([P, 4], F32, tag="recip", name="r")
                    nc.vector.reciprocal(out=r[:], in_=ys[:, :, D])
                    r_b = bass.AP(tensor=r.tensor, offset=r.offset,
                                  ap=[r.ap[0], r.ap[1], [0, D]])
                    nc.vector.tensor_mul(
                        ypre3[hp][:, :, hh, :], ys[:, :, 0:D], r_b)

                return [mk(kt) for kt in range(nkt)] + [fin]

            def proj_subunits(qc, ypre3, qs):
                """One row tile's projection as 4 filler closures."""
                nt_i = qc * 4 + qs
                box = {}

                def t_unit():
                    box["yt"] = yt_pool.tile([P, 3, P], BF16, tag="yt",
                                             name="yt")
                    box["osb"] = outsb_pool.tile([P, E], BF16, tag="osb",
                                                 name="osb")
                    box["po"] = ps_o.tile([P, 2, 256], F32, tag="ps_o",
                                          name="po")
                    b = next_slot3()
                    for hp in range(3):
                        nc.tensor.transpose(
                            psT[:, b + hp, :], ypre3[hp][:, qs, :, :], ident[:]
                        )
                    nc.vector.tensor_copy(box["yt"][:, :, :], psT[:, b:b + 3, :])

                def o_unit(oh):
                    def u():
                        yt, osb, po = box["yt"], box["osb"], box["po"]
                        sl = oh % 2
                        for ec in range(3):
                            nc.tensor.matmul(
                                po[:, sl, :],
                                yt[:, ec, :],
                                wt_sb[:, ec, oh * 256 : (oh + 1) * 256],
                                start=(ec == 0), stop=(ec == 2),
                            )
                        nc.vector.tensor_copy(
                            osb[:, oh * 256 : (oh + 1) * 256], po[:, sl, :]
                        )
                        if oh == 2:
                            nc.sync.dma_start(out=out_t[nt_i], in_=osb[:])
                    return u

                return [t_unit, o_unit(0), o_unit(1), o_unit(2)]

            # ---------------- main pipelined schedule ----------------
            pending_av = []     # at most one (h, qc, pts, ypre3)
            pending_proj = []   # (qc, ypre3, qs) row-tile units
            preprocess_startup()
            for qc in range(NQC):
                ypre3_cur = [
                    ypre_pool.tile([P, 4, 2, D], BF16, tag=f"ypre{_i}",
                                   name=f"ypre{_i}")
                    for _i in range(3)
                ]
                for h in range(HL):
                    if qc == 0 and h == 1:
                        # bulk loads after the startup-critical q/k DMAs:
                        # v first (first AV filler), then the remaining rope
                        # tables (group-1 finish), then weights (projection)
                        nc.sync.dma_start(out=vall[:], in_=v_ext[:])
                        nc.sync.dma_start(out=tabs_sb[:, 4:NT],
                                          in_=tabs_ext[:, 4 * TW:])
                    filler = []
                    meta = None
                    if pending_av:
                        ph, pqc, ppts, pypre = pending_av.pop()
                        filler = make_av_units(ph, pqc, ppts, pypre)
                        meta = (ph, pqc, pypre)
                    for _ in range(5):
                        if pending_proj:
                            filler.append(pending_proj.pop(0))
                    pts = emit_qk_exp(h, qc, filler, split_q=(h == 0))
                    for u in filler:
                        u()
                    if meta is not None:
                        ph, pqc, pypre = meta
                        if ph == HL - 1:
                            for qs in range(4):
                                pending_proj.extend(
                                    proj_subunits(pqc, pypre, qs))
                    pending_av.append((h, qc, pts, ypre3_cur))
                    # preprocess next q-chunk's tiles inside the head loop,
                    # k/q sides split across head windows to avoid bursts
                    # (qc0 starts at h==1 to keep startup uncongested)
                    if qc < NQC - 1:
                        hofs = 1 if qc == 0 else 0
                        if h == hofs:
                            grp = preprocess_group(4 * (qc + 1))
                        elif h == hofs + 1:
                            if qc == 0:
                                nc.sync.dma_start(out=wt_sb[:], in_=wt_ext[:])
                            finish_group_side(grp, 4 * (qc + 1), 0)
                        elif h == hofs + 2:
                            finish_group_side(grp, 4 * (qc + 1), 1)
            # drain
            ph, pqc, ppts, pypre = pending_av.pop()
            for u in make_av_units(ph, pqc, ppts, pypre):
                u()
            while pending_proj:
                pending_proj.pop(0)()
            for qs in range(4):
                for u in proj_subunits(pqc, pypre, qs):
                    u()
    return nc


def _get_graph():
    if "nc" not in _CACHE:
        _CACHE["nc"] = build_graph()
    return _CACHE["nc"]


def _host_inputs(q, k, v, q_scale, k_scale, proj_w):
    perm = _head_perm()
    bf = ml_dtypes.bfloat16
    qcos, qsin = _build_tables(q_scale, invert_xpos=False)
    kcos, ksin = _build_tables(k_scale, invert_xpos=True)
    # device computes rsqrt(ssum_q) = 0.125*rstd_q; fold the 8x here
    qcos, qsin = qcos * 8.0, qsin * 8.0

    def tab_layout(t):
        # [N, w] -> [128, NT*w] matching sbuf tile [P, NT, w]
        w = t.shape[1]
        return np.ascontiguousarray(
            t.reshape(NT, P, w).transpose(1, 0, 2).reshape(P, NT * w)).astype(bf)

    qcos_r, qsin_r = tab_layout(qcos), tab_layout(qsin)
    kcos_r, ksin_r = tab_layout(kcos), tab_layout(ksin)
    # combined [P, NT, 2, D+RDIM]: slot 0 = k tables, slot 1 = q tables
    tabs = np.empty((P, NT, 2, D + RDIM), dtype=kcos_r.dtype)
    tabs[:, :, 0, 0:D] = kcos_r.reshape(P, NT, D)
    tabs[:, :, 0, D:] = ksin_r.reshape(P, NT, RDIM)
    tabs[:, :, 1, 0:D] = qcos_r.reshape(P, NT, D)
    tabs[:, :, 1, D:] = qsin_r.reshape(P, NT, RDIM)
    tabs_r = np.ascontiguousarray(tabs.reshape(P, NT * TW))
    # additive causal mask for the diagonal [128,128] sub-block:
    # tri[k, q] = 0 where q >= k else -1e30 (exp -> exactly 0)
    tri = np.where(np.arange(P)[None, :] >= np.arange(P)[:, None],
                   0.0, -1e30).astype(np.float32)
    tri_r = np.ascontiguousarray(tri).astype(bf)

    in_maps = []
    for c in range(8):
        b = c // 2
        h0 = HL * (c % 2)
        cols = np.concatenate([(h0 + h) * D + perm for h in range(HL)])
        vcols = np.arange(h0 * D, (h0 + HL) * D)
        v_aug = np.ones((N, HL, D + 1), np.float32)
        v_aug[:, :, :D] = v[b][:, vcols].reshape(N, HL, D)
        wt_l = np.ascontiguousarray(proj_w[:, vcols].T)   # [384, 768]
        wt_r = np.ascontiguousarray(
            wt_l.reshape(3, P, E).transpose(1, 0, 2).reshape(P, 3 * E))
        in_maps.append({
            "q": np.ascontiguousarray(q[b][:, cols]).astype(bf),
            "k": np.ascontiguousarray(k[b][:, cols]).astype(bf),
            "v": np.ascontiguousarray(
                v_aug.reshape(NT, P, HL * (D + 1)).transpose(1, 0, 2)
                .reshape(P, NT * HL * (D + 1))).astype(bf),
            "wt": wt_r.astype(bf),
            "tabs": tabs_r,
            "tri": tri_r,
        })
    return in_maps


def kernel(q, k, v, q_scale, k_scale, proj_w, proj_b):
    nc = _get_graph()
    in_maps = _host_inputs(q, k, v, q_scale, k_scale, proj_w)
    res = run_bass_kernel_spmd(nc, in_maps, list(range(8)))
    out = np.empty((B, N, E), np.float32)
    for b in range(B):
        out[b] = (res.results[2 * b]["out"].astype(np.float32)
                  + res.results[2 * b + 1]["out"].astype(np.float32))
    out += proj_b[None, None, :].astype(np.float32)
    return out
